# revision 40
# baseline (speedup 1.0000x reference)
"""Bass/Trainium2 kernel for the 2-layer GAT (nn_GAT_11106785427688).

Strategy (8 NeuronCores, SPMD single NEFF):
- dst-ownership sharding: core c owns nodes [c*OWN, (c+1)*OWN); it receives
  every edge whose dst it owns (~137K edges), so segment-softmax denominators
  and message sums complete locally -- no all-reduce. One AllGather of the
  layer-1 activations between layers; host assembles the final output from
  per-core slices.
- Per-edge gather of packed [h | a_src.h] rows (fp16, 256B) from HBM tables
  via the SWDGE dma_gather custom op (int16 indices -> src buckets of BS
  rows; table rows permuted so the dense phase writes 2KB-contiguous runs).
- No indexed scatter (HW dma_scatter_add loses duplicate updates): edges are
  grouped by 128-node dst window; one-hot R [edges x nodes] (fp16) built on
  DVE via iota-compare turns segment-sum into PE matmul accumulated in PSUM.
  Softmax division is deferred: out = (sum_e w*h[src]) / (sum_e w).
- e = alpha_s[src] + alpha_d[dst] accumulates on PE: the RT@adw matmul adds
  the identity@alpha_s term into the same PSUM tile, so no DVE add.
- exp(leakyrelu(e)) = max(exp(e), exp(0.2 e)) -- two Exp activations share
  one act-func table (Lrelu lives in a different table set and would thrash).
- Layer 2 (1 head): the one-hot build folds the softmax weight in a dual-op
  TensorScalar ((iota==dst)*w), so messages need no elementwise multiply;
  the denominator comes from an extra ones-column matmul.
- The table is split into one DRAM tensor per src bucket so dense-phase
  writes of bucket b+1 overlap edge-sweep gathers of bucket b; per-window
  finals run as batched wide ops after each layer's sweep.
"""
import numpy as np
import ml_dtypes

from concourse import bacc, mybir
import concourse.tile as tile
from concourse.bass_utils import run_bass_kernel_spmd

# ---- problem constants ----
N = 100000
D = 64
H1, C1 = 4, 16
NEG = 0.2
NCORES = 8
OWN = 12544                 # 98 windows * 128 per core
BUCK = 32768
CHUNK = 1024                # gather idxs per dma_gather call (ring limit)
TPC = CHUNK // 128          # tiles per chunk = 8

F16 = mybir.dt.float16
F32 = mybir.dt.float32
BF16 = mybir.dt.bfloat16
I16 = mybir.dt.int16
NPF16 = np.float16
NPBF16 = ml_dtypes.bfloat16

AF = mybir.ActivationFunctionType
OP = mybir.AluOpType


def _derived():
    NW = OWN // 128
    NPAD = NCORES * OWN
    NBUCK = (NPAD + BUCK - 1) // BUCK
    TBL_ROWS = NBUCK * BUCK
    return NW, NPAD, NBUCK, TBL_ROWS


def _perm_row(src):
    """Permuted table row for node src: tb*1024 + p*8 + j (write-friendly)."""
    tb, r = np.divmod(src, 1024)
    j, p = np.divmod(r, 128)
    return tb * 1024 + p * 8 + j


def prep(edge_index):
    """Vectorized host prep. Returns (sched, idx_h, dcol_h, drow_h)."""
    NW, NPAD, NBUCK, TBL_ROWS = _derived()
    src = np.concatenate([np.asarray(edge_index[0]), np.arange(N, dtype=np.int64)])
    dst = np.concatenate([np.asarray(edge_index[1]), np.arange(N, dtype=np.int64)])
    owner = dst // OWN

    # balanced bucket width (1024-aligned for the table-row permutation)
    BS = ((NPAD + NBUCK - 1) // NBUCK + 1023) // 1024 * 1024
    assert BS <= BUCK
    per_core = []
    counts = np.zeros((NCORES, NBUCK * NW), np.int64)
    for c in range(NCORES):
        m = owner == c
        s = src[m]
        d = dst[m] - c * OWN
        w = d >> 7
        b = s // BS
        key = b * NW + w
        order = np.lexsort((s, key))
        s, d, key = s[order], d[order], key[order]
        per_core.append((s, d, key))
        counts[c] = np.bincount(key, minlength=NBUCK * NW)

    gsize = ((counts.max(0) + 127) // 128 * 128).astype(np.int64)  # [NBUCK*NW]
    gs2 = gsize.reshape(NBUCK, NW)
    live2 = gs2 > 0
    assert (live2.all(0) == live2.any(0)).all(), "window live in some buckets only"
    wlive = live2.any(0)
    slots_b = gs2.sum(1)
    slots_bp = (slots_b + CHUNK - 1) // CHUNK * CHUNK
    bucket_base = np.concatenate([[0], np.cumsum(slots_bp)])
    total_slots = int(bucket_base[-1])
    n_chunks = total_slots // CHUNK

    # group slot starts within the global layout
    gstart = np.zeros(NBUCK * NW, np.int64)
    for bb in range(NBUCK):
        gstart[bb * NW:(bb + 1) * NW] = bucket_base[bb] + np.concatenate(
            [[0], np.cumsum(gs2[bb][:-1])])

    # tile schedule
    n_tiles = total_slots // 128
    tile_w = np.full(n_tiles, -1, np.int64)
    tile_first = np.zeros(n_tiles, bool)
    tile_last = np.zeros(n_tiles, bool)
    for g in range(NBUCK * NW):
        t0 = gstart[g] // 128
        nt = gsize[g] // 128
        if nt == 0:
            continue
        tile_w[t0:t0 + nt] = g % NW
        tile_first[t0] = True
        tile_last[t0 + nt - 1] = True
    chunk_bucket = np.zeros(n_chunks, np.int64)
    for bb in range(NBUCK):
        chunk_bucket[bucket_base[bb] // CHUNK: bucket_base[bb + 1] // CHUNK] = bb

    # per-core slot arrays (vectorized)
    idx_h = np.zeros((NCORES, 128, n_chunks * (CHUNK // 16)), np.int16)
    dcol_h = np.zeros((NCORES, 128, n_chunks * TPC), np.float32)
    drow_h = np.zeros((NCORES, 1, n_chunks * CHUNK), NPBF16)
    for c in range(NCORES):
        s, d, key = per_core[c]
        grp_first = np.searchsorted(key, np.arange(NBUCK * NW))
        rank = np.arange(len(s)) - grp_first[key]
        slot = gstart[key] + rank
        gi = np.zeros(total_slots, np.int64)
        off = np.full(total_slots, -1, np.int64)
        pr = _perm_row(s)
        gi[slot] = pr - (s // BS) * BS
        off[slot] = d & 127
        assert (gi >= 0).all() and (gi < BS).all()
        # idx wrap: slot i of chunk ch at [i%16 + 16k, ch*64 + i//16]
        gia = gi.reshape(n_chunks, CHUNK // 16, 16).transpose(0, 2, 1)  # [ch,16,64]
        idx_h[c] = np.tile(gia, (1, 8, 1)).transpose(1, 0, 2).reshape(128, -1)
        # dcol: [p, ch*8+j] = off(ch*1024 + j*128 + p)
        offa = off.reshape(n_chunks, TPC, 128).transpose(2, 0, 1).reshape(128, -1)
        dcol_h[c] = offa.astype(np.float32)
        drow_h[c] = off.reshape(1, n_chunks * CHUNK).astype(NPBF16)

    sched = dict(n_chunks=n_chunks, tile_w=tile_w.tolist(),
                 tile_first=tile_first.tolist(), tile_last=tile_last.tolist(),
                 chunk_bucket=chunk_bucket.tolist(), bs=BS,
                 wlive=wlive.tolist())
    return sched, idx_h, dcol_h, drow_h


def build(sched, no_collective=False, debug=False, repeat=1):
    NW, NPAD, NBUCK, TBL_ROWS = _derived()
    n_chunks = sched["n_chunks"]
    tile_w = sched["tile_w"]
    tile_first = sched["tile_first"]
    tile_last = sched["tile_last"]
    chunk_bucket = sched["chunk_bucket"]
    BS = sched["bs"]
    wlive = sched["wlive"]
    NB_DENSE = (NPAD // 128 + 7) // 8          # 1024-row dense blocks
    BPB = BS // 1024                           # dense blocks per bucket
    EC1, EC2 = D + H1, D + 1
    WB = 14                                    # windows per batched-final block
    assert NW % WB == 0

    # chunk ranges per bucket
    bchunks = [[c for c in range(n_chunks) if chunk_bucket[c] == b]
               for b in range(NBUCK)]
    for b in range(NBUCK):
        assert bchunks[b] == list(range(bchunks[b][0], bchunks[b][-1] + 1))

    nc = bacc.Bacc(None, target_bir_lowering=False, num_swdge_queues=4)

    embT = nc.dram_tensor("embT", [D, NPAD], BF16, kind="ExternalInput")
    embTo = nc.dram_tensor("embTo", [D, OWN], BF16, kind="ExternalInput")
    w1aux = nc.dram_tensor("w1aux", [D, D + H1], BF16, kind="ExternalInput")
    w1ad = nc.dram_tensor("w1ad", [D, H1], BF16, kind="ExternalInput")
    w2aux = nc.dram_tensor("w2aux", [D, D + 1], BF16, kind="ExternalInput")
    w2ad = nc.dram_tensor("w2ad", [D, 1], BF16, kind="ExternalInput")
    b1t_in = nc.dram_tensor("b1t", [128, D], F32, kind="ExternalInput")
    b2t_in = nc.dram_tensor("b2t", [128, D], F32, kind="ExternalInput")
    iota_in = nc.dram_tensor("iotac", [128, 128], F16, kind="ExternalInput")
    pconst_in = nc.dram_tensor("pconst", [128, 1], F32, kind="ExternalInput")
    ident_in = nc.dram_tensor("ident", [128, 128], F32, kind="ExternalInput")
    identf_in = nc.dram_tensor("identf", [128, 128], F16, kind="ExternalInput")
    ones_in = nc.dram_tensor("ones1", [1, 128], BF16, kind="ExternalInput")
    idx_in = nc.dram_tensor("idx16", [128, n_chunks * (CHUNK // 16)], I16, kind="ExternalInput")
    dcol_in = nc.dram_tensor("dcol", [128, n_chunks * TPC], F32, kind="ExternalInput")
    drow_in = nc.dram_tensor("drow", [1, n_chunks * CHUNK], BF16, kind="ExternalInput")
    out_own = nc.dram_tensor("out_own", [OWN, D], F32, kind="ExternalOutput")
    if debug:
        dbg_adw = nc.dram_tensor("dbg_adw", [128, (OWN // 128) * H1], F16,
                                 kind="ExternalOutput")
        dbg_acc = [nc.dram_tensor(f"dbg_acc{b}", [128, (OWN // 128) * (D + H1)],
                                  BF16, kind="ExternalOutput")
                   for b in range(4)]
        dbg_ag = nc.dram_tensor("dbg_ag", [D, OWN], BF16, kind="ExternalOutput")
        dbg_acc2 = [nc.dram_tensor(f"dbg_acc2_{b}", [128, (OWN // 128) * (D + H1)],
                                   BF16, kind="ExternalOutput")
                    for b in range(4)]
        dbg_agout = nc.dram_tensor("dbg_agout", [NCORES * D, OWN], BF16,
                                   kind="ExternalOutput")

    # one table tensor per src bucket: lets bucket b+1's dense writes overlap
    # bucket b's gathers (tile deps are whole-tensor granular)
    tables = [nc.dram_tensor(f"table{b}", [BS, 128], F16) for b in range(NBUCK)]
    ag_in = nc.dram_tensor("ag_in", [D, OWN], BF16)
    ag_out = nc.dram_tensor("ag_out", [NCORES * D, OWN], BF16, addr_space="Shared")

    with tile.TileContext(nc) as tc:
        with tc.tile_pool(name="persist", bufs=1) as pp:
            b1t = pp.tile([128, D], F32)
            b2t = pp.tile([128, D], F32)
            iotac = pp.tile([128, 128], F16)
            pconst = pp.tile([128, 1], F32)
            ident = pp.tile([128, 128], F32)
            identf = pp.tile([128, 128], F16)
            ones1 = pp.tile([1, 128], BF16)
            w1x = pp.tile([D, D + H1], BF16)
            w1d = pp.tile([D, H1], BF16)
            w2x = pp.tile([D, D + 1], BF16)
            w2d = pp.tile([D, 1], BF16)
            idx_s = pp.tile([128, n_chunks * (CHUNK // 16)], I16)
            dcol_s = pp.tile([128, n_chunks * TPC], F32)
            adw = pp.tile([128, NW * H1], F16)
            adw2 = pp.tile([128, NW], F16)
            # per-(src bucket) partial segment sums, summed in batched finals
            accb = [pp.tile([128, NW * EC1], BF16, name=f"accb{i}")
                    for i in range(NBUCK)]
            for t_, s_ in [(b1t, b1t_in), (b2t, b2t_in), (iotac, iota_in),
                           (pconst, pconst_in), (ident, ident_in),
                           (identf, identf_in), (ones1, ones_in),
                           (w1x, w1aux), (w1d, w1ad), (w2x, w2aux), (w2d, w2ad),
                           (idx_s, idx_in), (dcol_s, dcol_in)]:
                nc.sync.dma_start(out=t_[:], in_=s_[:])

            def adw_fill(layer, ap, app):
                """Per-owned-window a_dst.h via x_own @ (W @ Ad)."""
                wad = w1d if layer == 1 else w2d
                H = H1 if layer == 1 else 1
                dst_t = adw if layer == 1 else adw2
                srcT = embTo if layer == 1 else ag_in
                xo = ap.tile([D, OWN], BF16, tag="xo")
                nc.sync.dma_start(out=xo[:], in_=srcT[:, :])
                for w0 in range(0, NW, 4):
                    nw_ = min(4, NW - w0)
                    ps = app.tile([128, 4 * H1], F32, tag="g")
                    for i in range(nw_):
                        w = w0 + i
                        nc.tensor.matmul(out=ps[:, i * H:(i + 1) * H],
                                         lhsT=xo[:, w * 128:(w + 1) * 128],
                                         rhs=wad[:], start=True, stop=True)
                    nc.scalar.activation(out=dst_t[:, w0 * H:(w0 + nw_) * H],
                                         in_=ps[:, 0:nw_ * H], func=AF.Copy)

            def dense_bucket(layer, b, dp, dpp):
                """x @ Waux -> fp16 rows of tables[b] (permuted layout).

                Table cols ncol..127 are never read, so stg is left unzeroed;
                PSUM->SBUF conversion runs on the Activation engine."""
                waux = w1x if layer == 1 else w2x
                ncol = EC1 if layer == 1 else EC2
                tb0, tb1 = b * BPB, min((b + 1) * BPB, NB_DENSE)
                for tb in range(tb0, tb1):
                    lt = dp.tile([D, 1024], BF16, tag="lhs")
                    if layer == 1:
                        nc.sync.dma_start(out=lt[:], in_=embT[:, tb * 1024:(tb + 1) * 1024])
                    else:
                        j = 0
                        while j < 8:
                            t = tb * 8 + j
                            co, wl = divmod(t, NW)
                            nrun = min(8 - j, NW - wl)
                            nc.sync.dma_start(
                                out=lt[:, j * 128:(j + nrun) * 128],
                                in_=ag_out[co * D:(co + 1) * D,
                                           wl * 128:(wl + nrun) * 128])
                            j += nrun
                    stg = dp.tile([128, 8 * 128], F16, tag="stg")
                    for half in range(2):
                        ps = dpp.tile([128, 4 * ncol], F32, tag="d")
                        for j in range(4):
                            nc.tensor.matmul(
                                out=ps[:, j * ncol:(j + 1) * ncol],
                                lhsT=lt[:, (half * 4 + j) * 128:(half * 4 + j + 1) * 128],
                                rhs=waux[:], start=True, stop=True)
                        nc.scalar.activation(
                            out=stg[:].rearrange("p (j k) -> p j k", k=128)[
                                :, half * 4:(half + 1) * 4, 0:ncol],
                            in_=ps[:].rearrange("p (j k) -> p j k", k=ncol),
                            func=AF.Copy)
                    r0 = tb * 1024 - b * BS
                    nc.sync.dma_start(
                        out=tables[b][r0:r0 + 1024].rearrange(
                            "(p j) k -> p (j k)", j=8),
                        in_=stg[:])

            def sweep_bucket(layer, b, pools, group_ps):
                gp, dp_, rp, mp, prp, pap, pgp = pools
                H = H1 if layer == 1 else 1
                EC = D + H
                adwl = adw if layer == 1 else adw2
                DRB = 8
                for c in bchunks[b]:
                    live = [j for j in range(TPC) if tile_w[c * TPC + j] >= 0]
                    assert live == list(range(len(live))), "pads must trail"
                    nl = len(live)
                    ght = gp.tile([128, TPC * 128], F16, tag="ght")
                    nc.gpsimd.dma_gather(
                        ght[:].rearrange("p (a k) -> p a k", k=128),
                        tables[b][:, :],
                        idx_s[:, c * (CHUNK // 16):(c + 1) * (CHUNK // 16)],
                        CHUNK, CHUNK, 128, elem_step=128, queue_num=c % 4)
                    if not live:
                        continue
                    ght3 = ght[:].rearrange("p (a k) -> p a k", k=128)
                    # replicate dstoff row via K=1 matmuls (bf16)
                    if (c - bchunks[b][0]) % DRB == 0:
                        nb = min(DRB, bchunks[b][-1] + 1 - c)
                        drt = dp_.tile([1, DRB * CHUNK], BF16, tag="drow")
                        nc.sync.dma_start(
                            out=drt[0:1, 0:nb * CHUNK],
                            in_=drow_in[0:1, c * CHUNK:(c + nb) * CHUNK])
                        sweep_bucket.drt = drt
                    drt = sweep_bucket.drt
                    doff = ((c - bchunks[b][0]) % DRB) * CHUNK
                    psr = prp.tile([128, CHUNK], F32, tag="r")
                    for hh in range(2):
                        nc.tensor.matmul(
                            out=psr[:, hh * 512:(hh + 1) * 512],
                            lhsT=ones1[:],
                            rhs=drt[0:1, doff + hh * 512:doff + (hh + 1) * 512],
                            start=True, stop=True)
                    RT = rp.tile([128, TPC * 128], F16, tag="RT")
                    nc.vector.tensor_tensor(
                        out=RT[:],
                        in0=pconst[:].to_broadcast([128, TPC * 128]),
                        in1=psr[:],
                        op=OP.is_equal)
                    # e = alpha_d[dst] + alpha_s[src], both terms on PE
                    psa = pap.tile([128, nl * H], F32, tag="a", name=f"psa{layer}_{c}")
                    for j in live:
                        w = tile_w[c * TPC + j]
                        nc.tensor.matmul(
                            out=psa[:, j * H:(j + 1) * H],
                            lhsT=RT[:, j * 128:(j + 1) * 128],
                            rhs=adwl[:, w * H:(w + 1) * H],
                            start=True, stop=False)
                        nc.tensor.matmul(
                            out=psa[:, j * H:(j + 1) * H],
                            lhsT=identf[:],
                            rhs=ght3[:, j, D:D + H],
                            start=False, stop=True)
                    # exp(leakyrelu(e)) = max(exp(e), exp(0.2 e))
                    z1 = mp.tile([128, nl * H], F16, tag="z1", name=f"z1_{layer}_{c}")
                    nc.scalar.activation(out=z1[:], in_=psa[:], func=AF.Exp)
                    z2 = mp.tile([128, nl * H], F16, tag="z2", name=f"z2_{layer}_{c}")
                    nc.scalar.activation(out=z2[:], in_=psa[:], scale=NEG, func=AF.Exp)
                    if layer == 1:
                        msgt = mp.tile([128, nl * EC], F16, tag="msg", name=f"msg1_{c}")
                        msgt3 = msgt[:].rearrange("p (a k) -> p a k", k=EC)
                        nc.vector.tensor_tensor(
                            out=msgt3[:, :, D:D + H],
                            in0=z1[:].rearrange("p (a h) -> p a h", h=H),
                            in1=z2[:].rearrange("p (a h) -> p a h", h=H),
                            op=OP.max)
                        R = rp.tile([128, TPC * 128], F16, tag="R")
                        for j in live:
                            nc.vector.tensor_scalar(
                                out=R[:, j * 128:(j + 1) * 128], in0=iotac[:],
                                scalar1=dcol_s[:, c * TPC + j:c * TPC + j + 1],
                                scalar2=None, op0=OP.is_equal)
                        meng = nc.gpsimd if c % 2 == 0 else nc.vector
                        meng.tensor_tensor(
                            out=msgt3[:, :, 0:D].rearrange("p a (h k) -> p a h k", k=C1),
                            in0=ght3[:, 0:nl, 0:D].rearrange("p a (h k) -> p a h k", k=C1),
                            in1=msgt3[:, :, D:D + H, None].to_broadcast(
                                [128, nl, H, C1]),
                            op=OP.mult)
                    else:
                        wt = mp.tile([128, nl], F32, tag="wt", name=f"wt_{c}")
                        nc.vector.tensor_tensor(out=wt[:], in0=z1[:], in1=z2[:],
                                                op=OP.max)
                        # fold softmax weight into the one-hot: (iota==dst)*w
                        R = rp.tile([128, TPC * 128], F16, tag="R")
                        for j in live:
                            nc.vector.tensor_scalar(
                                out=R[:, j * 128:(j + 1) * 128], in0=iotac[:],
                                scalar1=dcol_s[:, c * TPC + j:c * TPC + j + 1],
                                scalar2=wt[:, j:j + 1], op0=OP.is_equal,
                                op1=OP.mult)
                    for j in live:
                        t = c * TPC + j
                        w = tile_w[t]
                        if tile_first[t]:
                            group_ps[w] = pgp.tile([128, EC], F32, tag="g",
                                                   name=f"grp{layer}_{w}_{b}")
                        ps = group_ps[w]
                        if layer == 1:
                            nc.tensor.matmul(
                                out=ps[:], lhsT=R[:, j * 128:(j + 1) * 128],
                                rhs=msgt[:, j * EC:(j + 1) * EC],
                                start=tile_first[t], stop=tile_last[t])
                        else:
                            # num chain's start=True zeroes the whole PSUM
                            # bank, so the den chain must never re-start
                            nc.tensor.matmul(
                                out=ps[:, 0:D], lhsT=R[:, j * 128:(j + 1) * 128],
                                rhs=ght3[:, j, 0:D],
                                start=tile_first[t], stop=tile_last[t])
                            nc.tensor.matmul(
                                out=ps[:, D:D + 1], lhsT=R[:, j * 128:(j + 1) * 128],
                                rhs=iotac[:, 1:2],
                                start=False, stop=tile_last[t])
                        if tile_last[t]:
                            del group_ps[w]
                            nc.scalar.activation(
                                out=accb[b][:, w * EC:(w + 1) * EC],
                                in_=ps[:], func=AF.Copy)

            def clear_dead(EC):
                for w in range(NW):
                    if not wlive[w]:
                        for b in range(NBUCK):
                            nc.vector.memset(accb[b][:, w * EC:(w + 1) * EC], 0.0)

            def f1_batch(fp, fpp):
                """Softmax divide + bias + elu + transpose, WB windows at once."""
                EC = EC1
                for w0 in range(0, NW, WB):
                    cs = slice(w0 * EC, (w0 + WB) * EC)
                    sm = fp.tile([128, WB * EC], F32, tag="sm")
                    nc.vector.tensor_tensor(out=sm[:], in0=accb[0][:, cs],
                                            in1=accb[1][:, cs], op=OP.add)
                    nc.vector.tensor_tensor(out=sm[:], in0=sm[:], in1=accb[2][:, cs],
                                            op=OP.add)
                    nc.vector.tensor_tensor(out=sm[:], in0=sm[:], in1=accb[3][:, cs],
                                            op=OP.add)
                    sm3 = sm[:].rearrange("p (a k) -> p a k", k=EC)
                    den = fp.tile([128, WB * H1], F32, tag="den")
                    nc.vector.tensor_scalar_add(
                        out=den[:].rearrange("p (a h) -> p a h", h=H1),
                        in0=sm3[:, :, D:EC], scalar1=1e-16)
                    rec = fp.tile([128, WB * H1], F32, tag="rec")
                    nc.vector.reciprocal(out=rec[:], in_=den[:])
                    x2 = fp.tile([128, WB * D], F32, tag="x2")
                    nc.vector.tensor_tensor(
                        out=x2[:].rearrange("p (a h k) -> p a h k", h=H1, k=C1),
                        in0=sm3[:, :, 0:D].rearrange("p a (h k) -> p a h k", k=C1),
                        in1=rec[:].rearrange("p (a h) -> p a h", h=H1)[:, :, :, None]
                            .to_broadcast([128, WB, H1, C1]),
                        op=OP.mult)
                    nc.vector.tensor_tensor(
                        out=x2[:].rearrange("p (a k) -> p a k", k=D),
                        in0=x2[:].rearrange("p (a k) -> p a k", k=D),
                        in1=b1t[:, None, :].to_broadcast([128, WB, D]),
                        op=OP.add)
                    # elu(x) = relu(x) - relu(1 - exp(x))
                    ex = fp.tile([128, WB * D], F32, tag="ex")
                    nc.scalar.activation(out=ex[:], in_=x2[:], func=AF.Exp)
                    u = fp.tile([128, WB * D], F32, tag="u")
                    nc.scalar.activation(out=u[:], in_=ex[:], func=AF.Relu,
                                         scale=-1.0, bias=1.0)
                    r = fp.tile([128, WB * D], F32, tag="ex")
                    nc.scalar.activation(out=r[:], in_=x2[:], func=AF.Relu)
                    xe = fp.tile([128, WB * D], F32, tag="x2")
                    nc.vector.tensor_tensor(out=xe[:], in0=r[:], in1=u[:],
                                            op=OP.subtract)
                    xt = fp.tile([D, WB * 128], BF16, tag="xt")
                    for a in range(WB):
                        pst = fpp.tile([D, 128], F32, tag="t")
                        nc.tensor.transpose(out=pst[:], in_=xe[:, a * D:(a + 1) * D],
                                            identity=ident[:])
                        nc.scalar.activation(out=xt[:, a * 128:(a + 1) * 128],
                                             in_=pst[:], func=AF.Copy)
                    nc.sync.dma_start(
                        out=ag_in[:, w0 * 128:(w0 + WB) * 128], in_=xt[:])

            def f2_batch(fp):
                """Softmax divide + bias + l2-normalize, WB windows at once."""
                EC = EC2
                for w0 in range(0, NW, WB):
                    cs = slice(w0 * EC, (w0 + WB) * EC)
                    sm = fp.tile([128, WB * EC], F32, tag="sm")
                    nc.vector.tensor_tensor(out=sm[:], in0=accb[0][:, cs],
                                            in1=accb[1][:, cs], op=OP.add)
                    nc.vector.tensor_tensor(out=sm[:], in0=sm[:], in1=accb[2][:, cs],
                                            op=OP.add)
                    nc.vector.tensor_tensor(out=sm[:], in0=sm[:], in1=accb[3][:, cs],
                                            op=OP.add)
                    sm3 = sm[:].rearrange("p (a k) -> p a k", k=EC)
                    den = fp.tile([128, WB], F32, tag="den")
                    nc.vector.tensor_scalar_add(
                        out=den[:, :, None], in0=sm3[:, :, D:EC], scalar1=1e-16)
                    rec = fp.tile([128, WB], F32, tag="rec")
                    nc.vector.reciprocal(out=rec[:], in_=den[:])
                    o = fp.tile([128, WB * D], F32, tag="o")
                    nc.vector.tensor_tensor(
                        out=o[:].rearrange("p (a k) -> p a k", k=D),
                        in0=sm3[:, :, 0:D],
                        in1=rec[:, :, None].to_broadcast([128, WB, D]),
                        op=OP.mult)
                    nc.vector.tensor_tensor(
                        out=o[:].rearrange("p (a k) -> p a k", k=D),
                        in0=o[:].rearrange("p (a k) -> p a k", k=D),
                        in1=b2t[:, None, :].to_broadcast([128, WB, D]),
                        op=OP.add)
                    sq = fp.tile([128, WB * D], F32, tag="sq")
                    nc.scalar.activation(out=sq[:], in_=o[:], func=AF.Square)
                    ss = fp.tile([128, WB], F32, tag="ss")
                    nc.vector.tensor_reduce(
                        out=ss[:, :, None],
                        in_=sq[:].rearrange("p (a k) -> p a k", k=D),
                        axis=mybir.AxisListType.X, op=OP.add)
                    nrm = fp.tile([128, WB], F32, tag="nr")
                    nc.scalar.activation(out=nrm[:], in_=ss[:], func=AF.Sqrt)
                    nc.vector.tensor_scalar_max(out=nrm[:], in0=nrm[:], scalar1=1e-12)
                    rn = fp.tile([128, WB], F32, tag="rn")
                    nc.vector.reciprocal(out=rn[:], in_=nrm[:])
                    of = fp.tile([128, WB * D], F32, tag="of")
                    nc.vector.tensor_tensor(
                        out=of[:].rearrange("p (a k) -> p a k", k=D),
                        in0=o[:].rearrange("p (a k) -> p a k", k=D),
                        in1=rn[:, :, None].to_broadcast([128, WB, D]),
                        op=OP.mult)
                    nc.sync.dma_start(
                        out=out_own[w0 * 128:(w0 + WB) * 128, :].rearrange(
                            "(a p) k -> p a k", p=128),
                        in_=of[:].rearrange("p (a k) -> p a k", k=D))

            def one_pass(_rep):
                # ================= layer 1 =================
                with tc.tile_pool(name=f"l1_{_rep}", bufs=3) as dp, \
                     tc.tile_pool(name=f"l1p_{_rep}", bufs=1, space="PSUM") as dpp, \
                     tc.tile_pool(name=f"eg1_{_rep}", bufs=6) as gp, \
                     tc.tile_pool(name=f"ed1_{_rep}", bufs=2) as dp_, \
                     tc.tile_pool(name=f"er1_{_rep}", bufs=4) as rp, \
                     tc.tile_pool(name=f"em1_{_rep}", bufs=6) as mp, \
                     tc.tile_pool(name=f"epr1_{_rep}", bufs=1, space="PSUM") as prp, \
                     tc.tile_pool(name=f"epa1_{_rep}", bufs=1, space="PSUM") as pap, \
                     tc.tile_pool(name=f"epg1_{_rep}", bufs=2, space="PSUM") as pgp, \
                     tc.tile_pool(name=f"f1p_{_rep}", bufs=2, space="PSUM") as fpp:
                    with tc.tile_pool(name=f"xo1_{_rep}", bufs=1) as xop:
                        adw_fill(1, xop, pgp)
                    pools = (gp, dp_, rp, mp, prp, pap, pgp)
                    group_ps = {}
                    for b in range(NBUCK):
                        dense_bucket(1, b, dp, dpp)
                        sweep_bucket(1, b, pools, group_ps)
                    assert not group_ps
                    clear_dead(EC1)
                    if debug:
                        nc.sync.dma_start(out=dbg_adw[:], in_=adw[:])
                        for b in range(NBUCK):
                            nc.sync.dma_start(out=dbg_acc[b][:], in_=accb[b][:])
                    with tc.tile_pool(name=f"f1_{_rep}", bufs=2) as fp:
                        f1_batch(fp, fpp)
                    if debug:
                        with tc.tile_pool(name=f"dbgag_{_rep}", bufs=1) as dap:
                            agt = dap.tile([D, OWN], BF16, tag="agt")
                            nc.sync.dma_start(out=agt[:], in_=ag_in[:])
                            nc.sync.dma_start(out=dbg_ag[:], in_=agt[:])
                    # adw for layer 2 needs only own ag_in -- overlap with f1
                    with tc.tile_pool(name=f"xo2_{_rep}", bufs=1) as xop2:
                        adw_fill(2, xop2, pgp)

                if no_collective:
                    # profiling-only variant: local copy stands in for AllGather
                    with tc.tile_pool(name=f"agcp_{_rep}", bufs=2) as acp:
                        for cc in range(NCORES):
                            t_ = acp.tile([D, OWN], BF16, tag="agc")
                            nc.sync.dma_start(out=t_[:], in_=ag_in[:])
                            nc.sync.dma_start(out=ag_out[cc * D:(cc + 1) * D, :],
                                              in_=t_[:])
                else:
                    nc.gpsimd.collective_compute(
                        "AllGather", OP.bypass,
                        ins=[ag_in[:]], outs=[ag_out[:]],
                        replica_groups=[list(range(NCORES))])

                # ================= layer 2 =================
                with tc.tile_pool(name=f"l2_{_rep}", bufs=3) as dp, \
                     tc.tile_pool(name=f"l2p_{_rep}", bufs=1, space="PSUM") as dpp, \
                     tc.tile_pool(name=f"eg2_{_rep}", bufs=6) as gp, \
                     tc.tile_pool(name=f"ed2_{_rep}", bufs=2) as dp_, \
                     tc.tile_pool(name=f"er2_{_rep}", bufs=4) as rp, \
                     tc.tile_pool(name=f"em2_{_rep}", bufs=6) as mp, \
                     tc.tile_pool(name=f"epr2_{_rep}", bufs=1, space="PSUM") as prp, \
                     tc.tile_pool(name=f"epa2_{_rep}", bufs=1, space="PSUM") as pap, \
                     tc.tile_pool(name=f"epg2_{_rep}", bufs=2, space="PSUM") as pgp:
                    pools = (gp, dp_, rp, mp, prp, pap, pgp)
                    group_ps = {}
                    for b in range(NBUCK):
                        dense_bucket(2, b, dp, dpp)
                        sweep_bucket(2, b, pools, group_ps)
                    assert not group_ps
                    clear_dead(EC2)
                    if debug:
                        for b in range(NBUCK):
                            nc.sync.dma_start(out=dbg_acc2[b][:], in_=accb[b][:])
                        with tc.tile_pool(name=f"dbgao_{_rep}", bufs=2) as dap:
                            for cc in range(NCORES):
                                aot = dap.tile([D, OWN], BF16, tag="aot")
                                nc.sync.dma_start(out=aot[:],
                                                  in_=ag_out[cc * D:(cc + 1) * D, :])
                                nc.sync.dma_start(
                                    out=dbg_agout[cc * D:(cc + 1) * D, :],
                                    in_=aot[:])
                    with tc.tile_pool(name=f"f2_{_rep}", bufs=2) as fp:
                        f2_batch(fp)

            for _rep in range(repeat):
                one_pass(_rep)

    return nc


def make_inputs(edge_index, emb, W1, a_src1, a_dst1, b1, W2, a_src2, a_dst2, b2):
    NW, NPAD, NBUCK, TBL_ROWS = _derived()
    sched, idx_h, dcol_h, drow_h = prep(edge_index)

    W1 = np.asarray(W1, np.float32)
    a_s1 = np.asarray(a_src1, np.float32)
    a_d1 = np.asarray(a_dst1, np.float32)
    As = np.zeros((D, H1), np.float32)
    Ad = np.zeros((D, H1), np.float32)
    for h in range(H1):
        As[h * C1:(h + 1) * C1, h] = a_s1[h]
        Ad[h * C1:(h + 1) * C1, h] = a_d1[h]
    w1x = np.concatenate([W1, W1 @ As], 1).astype(NPBF16)
    w1d = (W1 @ Ad).astype(NPBF16)
    W2 = np.asarray(W2, np.float32)
    w2x = np.concatenate([W2, W2 @ np.asarray(a_src2, np.float32).T], 1).astype(NPBF16)
    w2d = (W2 @ np.asarray(a_dst2, np.float32).T).astype(NPBF16)

    embT = np.zeros((D, NPAD), NPBF16)
    embT[:, :N] = np.asarray(emb, np.float32).T.astype(NPBF16)
    iotac = np.broadcast_to(np.arange(128, dtype=NPF16)[None, :], (128, 128)).copy()
    pconst = np.arange(128, dtype=np.float32)[:, None].copy()
    ident = np.eye(128, dtype=np.float32)
    identf = np.eye(128, dtype=NPF16)
    ones1 = np.ones((1, 128), NPBF16)
    b1t = np.broadcast_to(np.asarray(b1, np.float32)[None, :], (128, D)).copy()
    b2t = np.broadcast_to(np.asarray(b2, np.float32)[None, :], (128, D)).copy()

    in_maps = []
    for c in range(NCORES):
        in_maps.append({
            "embT": embT, "embTo": np.ascontiguousarray(embT[:, c * OWN:(c + 1) * OWN]),
            "w1aux": w1x, "w1ad": w1d, "w2aux": w2x, "w2ad": w2d,
            "b1t": b1t, "b2t": b2t, "iotac": iotac, "pconst": pconst,
            "ident": ident, "identf": identf, "ones1": ones1,
            "idx16": idx_h[c], "dcol": dcol_h[c], "drow": drow_h[c],
        })
    return sched, in_maps


def kernel(edge_index, emb, W1, a_src1, a_dst1, b1, W2, a_src2, a_dst2, b2):
    sched, in_maps = make_inputs(edge_index, emb, W1, a_src1, a_dst1, b1,
                                 W2, a_src2, a_dst2, b2)
    nc = build(sched)
    nc.finalize()
    res = run_bass_kernel_spmd(nc, in_maps, core_ids=list(range(NCORES)))
    out = np.zeros((N, D), np.float32)
    for c in range(NCORES):
        lo, hi = c * OWN, min((c + 1) * OWN, N)
        if lo < N:
            out[lo:hi] = res.results[c]["out_own"][:hi - lo]
    return out


# revision 44
# speedup vs baseline: 1.8064x; 1.8064x over previous
"""Bass/Trainium2 kernel for the 2-layer GAT (nn_GAT_11106785427688).

Strategy (8 NeuronCores, SPMD single NEFF):
- dst-ownership sharding: core c owns nodes [c*OWN, (c+1)*OWN); it receives
  every edge whose dst it owns (~137K edges), so segment-softmax denominators
  and message sums complete locally -- no all-reduce. One AllGather of the
  layer-1 activations between layers; host assembles the final output from
  per-core slices.
- Per-edge gather of packed [h | a_src.h] rows (fp16, 256B) from HBM tables
  via the SWDGE dma_gather custom op (int16 indices -> src buckets of BS
  rows; table rows permuted so the dense phase writes 2KB-contiguous runs).
- No indexed scatter (HW dma_scatter_add loses duplicate updates): edges are
  grouped by 128-node dst window; one-hot R [edges x nodes] (fp16) built on
  DVE via iota-compare turns segment-sum into PE matmul accumulated in PSUM.
  Softmax division is deferred: out = (sum_e w*h[src]) / (sum_e w).
- exp(leakyrelu(e)) = max(exp(e), exp(0.2 e)) -- two Exp activations share
  one act-func table (Lrelu lives in a different table set and would thrash).
- Layer 2 (1 head): the one-hot build folds the softmax weight in a dual-op
  TensorScalar ((iota==dst)*w); the consumed alpha_s column of the gather is
  overwritten with ones so a single matmul yields [sum w*h | sum w].
- The table is split into one DRAM tensor per src bucket so dense-phase
  writes of bucket b+1 overlap edge-sweep gathers of bucket b; per-window
  finals run as batched wide ops after each layer's sweep.
"""
import numpy as np
import ml_dtypes

from concourse import bacc, mybir
import concourse.tile as tile
from concourse.bass_utils import run_bass_kernel_spmd

# ---- problem constants ----
N = 100000
D = 64
H1, C1 = 4, 16
NEG = 0.2
NCORES = 8
OWN = 12544                 # 98 windows * 128 per core
BUCK = 32768
CHUNK = 1024                # gather idxs per dma_gather call (ring limit)
TPC = CHUNK // 128          # tiles per chunk = 8

F16 = mybir.dt.float16
F32 = mybir.dt.float32
BF16 = mybir.dt.bfloat16
I16 = mybir.dt.int16
NPF16 = np.float16
NPBF16 = ml_dtypes.bfloat16

AF = mybir.ActivationFunctionType
OP = mybir.AluOpType


def _derived():
    NW = OWN // 128
    NPAD = NCORES * OWN
    NBUCK = (NPAD + BUCK - 1) // BUCK
    TBL_ROWS = NBUCK * BUCK
    return NW, NPAD, NBUCK, TBL_ROWS


def _perm_row(src):
    """Permuted table row for node src: tb*1024 + p*8 + j (write-friendly)."""
    tb, r = np.divmod(src, 1024)
    j, p = np.divmod(r, 128)
    return tb * 1024 + p * 8 + j


def prep(edge_index):
    """Vectorized host prep. Returns (sched, idx_h, dcol_h, drow_h)."""
    NW, NPAD, NBUCK, TBL_ROWS = _derived()
    src = np.concatenate([np.asarray(edge_index[0]), np.arange(N, dtype=np.int64)])
    dst = np.concatenate([np.asarray(edge_index[1]), np.arange(N, dtype=np.int64)])
    owner = dst // OWN

    # balanced bucket width (1024-aligned for the table-row permutation)
    BS = ((NPAD + NBUCK - 1) // NBUCK + 1023) // 1024 * 1024
    assert BS <= BUCK
    per_core = []
    counts = np.zeros((NCORES, NBUCK * NW), np.int64)
    for c in range(NCORES):
        m = owner == c
        s = src[m]
        d = dst[m] - c * OWN
        w = d >> 7
        b = s // BS
        key = b * NW + w
        order = np.lexsort((s, key))
        s, d, key = s[order], d[order], key[order]
        per_core.append((s, d, key))
        counts[c] = np.bincount(key, minlength=NBUCK * NW)

    gsize = ((counts.max(0) + 127) // 128 * 128).astype(np.int64)  # [NBUCK*NW]
    gs2 = gsize.reshape(NBUCK, NW)
    live2 = gs2 > 0
    assert (live2.all(0) == live2.any(0)).all(), "window live in some buckets only"
    wlive = live2.any(0)
    slots_b = gs2.sum(1)
    slots_bp = (slots_b + CHUNK - 1) // CHUNK * CHUNK
    bucket_base = np.concatenate([[0], np.cumsum(slots_bp)])
    total_slots = int(bucket_base[-1])
    n_chunks = total_slots // CHUNK

    # group slot starts within the global layout
    gstart = np.zeros(NBUCK * NW, np.int64)
    for bb in range(NBUCK):
        gstart[bb * NW:(bb + 1) * NW] = bucket_base[bb] + np.concatenate(
            [[0], np.cumsum(gs2[bb][:-1])])

    # tile schedule
    n_tiles = total_slots // 128
    tile_w = np.full(n_tiles, -1, np.int64)
    tile_first = np.zeros(n_tiles, bool)
    tile_last = np.zeros(n_tiles, bool)
    for g in range(NBUCK * NW):
        t0 = gstart[g] // 128
        nt = gsize[g] // 128
        if nt == 0:
            continue
        tile_w[t0:t0 + nt] = g % NW
        tile_first[t0] = True
        tile_last[t0 + nt - 1] = True
    chunk_bucket = np.zeros(n_chunks, np.int64)
    for bb in range(NBUCK):
        chunk_bucket[bucket_base[bb] // CHUNK: bucket_base[bb + 1] // CHUNK] = bb

    # per-core slot arrays (vectorized)
    idx_h = np.zeros((NCORES, 128, n_chunks * (CHUNK // 16)), np.int16)
    dcol_h = np.zeros((NCORES, 128, n_chunks * TPC), np.float32)
    drow_h = np.zeros((NCORES, 1, n_chunks * CHUNK), NPBF16)
    for c in range(NCORES):
        s, d, key = per_core[c]
        grp_first = np.searchsorted(key, np.arange(NBUCK * NW))
        rank = np.arange(len(s)) - grp_first[key]
        slot = gstart[key] + rank
        gi = np.zeros(total_slots, np.int64)
        off = np.full(total_slots, -1, np.int64)
        pr = _perm_row(s)
        gi[slot] = pr - (s // BS) * BS
        off[slot] = d & 127
        assert (gi >= 0).all() and (gi < BS).all()
        # idx wrap: slot i of chunk ch at [i%16 + 16k, ch*64 + i//16]
        gia = gi.reshape(n_chunks, CHUNK // 16, 16).transpose(0, 2, 1)  # [ch,16,64]
        idx_h[c] = np.tile(gia, (1, 8, 1)).transpose(1, 0, 2).reshape(128, -1)
        # dcol: [p, ch*8+j] = off(ch*1024 + j*128 + p)
        offa = off.reshape(n_chunks, TPC, 128).transpose(2, 0, 1).reshape(128, -1)
        dcol_h[c] = offa.astype(np.float32)
        drow_h[c] = off.reshape(1, n_chunks * CHUNK).astype(NPBF16)

    sched = dict(n_chunks=n_chunks, tile_w=tile_w.tolist(),
                 tile_first=tile_first.tolist(), tile_last=tile_last.tolist(),
                 chunk_bucket=chunk_bucket.tolist(), bs=BS,
                 wlive=wlive.tolist())
    return sched, idx_h, dcol_h, drow_h


def build(sched, no_collective=False, debug=False, repeat=1,
          ew_pe=False, r_tsp=False, msg_pool=False):
    NW, NPAD, NBUCK, TBL_ROWS = _derived()
    n_chunks = sched["n_chunks"]
    tile_w = sched["tile_w"]
    tile_first = sched["tile_first"]
    tile_last = sched["tile_last"]
    chunk_bucket = sched["chunk_bucket"]
    BS = sched["bs"]
    wlive = sched["wlive"]
    NB_DENSE = (NPAD // 128 + 7) // 8          # 1024-row dense blocks
    BPB = BS // 1024                           # dense blocks per bucket
    EC1, EC2 = D + H1, D + 1
    WB = 14                                    # windows per batched-final block
    assert NW % WB == 0

    # chunk ranges per bucket
    bchunks = [[c for c in range(n_chunks) if chunk_bucket[c] == b]
               for b in range(NBUCK)]
    for b in range(NBUCK):
        assert bchunks[b] == list(range(bchunks[b][0], bchunks[b][-1] + 1))

    nc = bacc.Bacc(None, target_bir_lowering=False, num_swdge_queues=4)

    embT = nc.dram_tensor("embT", [D, NPAD], BF16, kind="ExternalInput")
    embTo = nc.dram_tensor("embTo", [D, OWN], BF16, kind="ExternalInput")
    w1aux = nc.dram_tensor("w1aux", [D, D + H1], BF16, kind="ExternalInput")
    w1ad = nc.dram_tensor("w1ad", [D, H1], BF16, kind="ExternalInput")
    w2aux = nc.dram_tensor("w2aux", [D, D + 1], BF16, kind="ExternalInput")
    w2ad = nc.dram_tensor("w2ad", [D, 1], BF16, kind="ExternalInput")
    b1t_in = nc.dram_tensor("b1t", [128, D], F32, kind="ExternalInput")
    b2t_in = nc.dram_tensor("b2t", [128, D], F32, kind="ExternalInput")
    iota_in = nc.dram_tensor("iotac", [128, 128], F16, kind="ExternalInput")
    pconst_in = nc.dram_tensor("pconst", [128, 1], F32, kind="ExternalInput")
    ident_in = nc.dram_tensor("ident", [128, 128], F32, kind="ExternalInput")
    identf_in = nc.dram_tensor("identf", [128, 128], F16, kind="ExternalInput")
    ones_in = nc.dram_tensor("ones1", [1, 128], BF16, kind="ExternalInput")
    idx_in = nc.dram_tensor("idx16", [128, n_chunks * (CHUNK // 16)], I16, kind="ExternalInput")
    dcol_in = nc.dram_tensor("dcol", [128, n_chunks * TPC], F32, kind="ExternalInput")
    dcolh_in = nc.dram_tensor("dcolh", [128, n_chunks * TPC], F16, kind="ExternalInput")
    drow_in = nc.dram_tensor("drow", [1, n_chunks * CHUNK], BF16, kind="ExternalInput")
    out_own = nc.dram_tensor("out_own", [OWN, D], F32, kind="ExternalOutput")
    if debug:
        dbg_adw = nc.dram_tensor("dbg_adw", [128, (OWN // 128) * H1], F16,
                                 kind="ExternalOutput")
        dbg_acc = [nc.dram_tensor(f"dbg_acc{b}", [128, (OWN // 128) * (D + H1)],
                                  BF16, kind="ExternalOutput")
                   for b in range(4)]
        dbg_ag = nc.dram_tensor("dbg_ag", [D, OWN], BF16, kind="ExternalOutput")
        dbg_acc2 = [nc.dram_tensor(f"dbg_acc2_{b}", [128, (OWN // 128) * (D + H1)],
                                   BF16, kind="ExternalOutput")
                    for b in range(4)]
        dbg_agout = nc.dram_tensor("dbg_agout", [NCORES * D, OWN], BF16,
                                   kind="ExternalOutput")

    # one table tensor per src bucket: lets bucket b+1's dense writes overlap
    # bucket b's gathers (tile deps are whole-tensor granular)
    tables = [nc.dram_tensor(f"table{b}", [BS, 128], F16) for b in range(NBUCK)]
    ag_in = nc.dram_tensor("ag_in", [D, OWN], BF16)
    ag_out = nc.dram_tensor("ag_out", [NCORES * D, OWN], BF16, addr_space="Shared")

    with tile.TileContext(nc) as tc:
        with tc.tile_pool(name="persist", bufs=1) as pp:
            b1t = pp.tile([128, D], F32)
            b2t = pp.tile([128, D], F32)
            iotac = pp.tile([128, 128], F16)
            pconst = pp.tile([128, 1], F32)
            ident = pp.tile([128, 128], F32)
            identf = pp.tile([128, 128], F16)
            ones1 = pp.tile([1, 128], BF16)
            w1x = pp.tile([D, D + H1], BF16)
            w1d = pp.tile([D, H1], BF16)
            w2x = pp.tile([D, D + 1], BF16)
            w2d = pp.tile([D, 1], BF16)
            idx_s = pp.tile([128, n_chunks * (CHUNK // 16)], I16)
            dcol_s = pp.tile([128, n_chunks * TPC], F32)
            dcolh_s = pp.tile([128, n_chunks * TPC], F16)
            adw = pp.tile([128, NW * H1], F16)
            adw2 = pp.tile([128, NW], F16)
            # per-(src bucket) partial segment sums, summed in batched finals
            accb = [pp.tile([128, NW * EC1], BF16, name=f"accb{i}")
                    for i in range(NBUCK)]
            for t_, s_ in [(b1t, b1t_in), (b2t, b2t_in), (iotac, iota_in),
                           (pconst, pconst_in), (ident, ident_in),
                           (identf, identf_in), (ones1, ones_in),
                           (w1x, w1aux), (w1d, w1ad), (w2x, w2aux), (w2d, w2ad),
                           (idx_s, idx_in), (dcol_s, dcol_in),
                           (dcolh_s, dcolh_in)]:
                nc.sync.dma_start(out=t_[:], in_=s_[:])

            def adw_fill(layer, ap, app):
                """Per-owned-window a_dst.h via x_own @ (W @ Ad)."""
                wad = w1d if layer == 1 else w2d
                H = H1 if layer == 1 else 1
                dst_t = adw if layer == 1 else adw2
                srcT = embTo if layer == 1 else ag_in
                xo = ap.tile([D, OWN], BF16, tag="xo")
                nc.sync.dma_start(out=xo[:], in_=srcT[:, :])
                for w0 in range(0, NW, 4):
                    nw_ = min(4, NW - w0)
                    ps = app.tile([128, 4 * H1], F32, tag="g")
                    for i in range(nw_):
                        w = w0 + i
                        nc.tensor.matmul(out=ps[:, i * H:(i + 1) * H],
                                         lhsT=xo[:, w * 128:(w + 1) * 128],
                                         rhs=wad[:], start=True, stop=True)
                    nc.scalar.activation(out=dst_t[:, w0 * H:(w0 + nw_) * H],
                                         in_=ps[:, 0:nw_ * H], func=AF.Copy)

            def dense_bucket(layer, b, dp, dpp):
                """x @ Waux -> fp16 rows of tables[b] (permuted layout).

                Table cols ncol..127 are never read, so stg is left unzeroed;
                PSUM->SBUF conversion runs on the Activation engine."""
                waux = w1x if layer == 1 else w2x
                ncol = EC1 if layer == 1 else EC2
                tb0, tb1 = b * BPB, min((b + 1) * BPB, NB_DENSE)
                for tb in range(tb0, tb1):
                    lt = dp.tile([D, 1024], BF16, tag="lhs")
                    if layer == 1:
                        nc.sync.dma_start(out=lt[:], in_=embT[:, tb * 1024:(tb + 1) * 1024])
                    else:
                        j = 0
                        while j < 8:
                            t = tb * 8 + j
                            co, wl = divmod(t, NW)
                            nrun = min(8 - j, NW - wl)
                            nc.sync.dma_start(
                                out=lt[:, j * 128:(j + nrun) * 128],
                                in_=ag_out[co * D:(co + 1) * D,
                                           wl * 128:(wl + nrun) * 128])
                            j += nrun
                    stg = dp.tile([128, 8 * 128], F16, tag="stg")
                    for half in range(2):
                        ps = dpp.tile([128, 4 * ncol], F32, tag="d")
                        for j in range(4):
                            nc.tensor.matmul(
                                out=ps[:, j * ncol:(j + 1) * ncol],
                                lhsT=lt[:, (half * 4 + j) * 128:(half * 4 + j + 1) * 128],
                                rhs=waux[:], start=True, stop=True)
                        nc.scalar.activation(
                            out=stg[:].rearrange("p (j k) -> p j k", k=128)[
                                :, half * 4:(half + 1) * 4, 0:ncol],
                            in_=ps[:].rearrange("p (j k) -> p j k", k=ncol),
                            func=AF.Copy)
                    r0 = tb * 1024 - b * BS
                    nc.sync.dma_start(
                        out=tables[b][r0:r0 + 1024].rearrange(
                            "(p j) k -> p (j k)", j=8),
                        in_=stg[:])

            def sweep_bucket(layer, b, pools, group_ps):
                gp, dp_, rp, mp, prp, pap, pgp = pools
                H = H1 if layer == 1 else 1
                EC = D + H
                adwl = adw if layer == 1 else adw2
                DRB = 8
                for c in bchunks[b]:
                    live = [j for j in range(TPC) if tile_w[c * TPC + j] >= 0]
                    assert live == list(range(len(live))), "pads must trail"
                    nl = len(live)
                    ght = gp.tile([128, TPC * 128], F16, tag="ght")
                    nc.gpsimd.dma_gather(
                        ght[:].rearrange("p (a k) -> p a k", k=128),
                        tables[b][:, :],
                        idx_s[:, c * (CHUNK // 16):(c + 1) * (CHUNK // 16)],
                        CHUNK, CHUNK, 128, elem_step=128, queue_num=c % 4)
                    if not live:
                        continue
                    ght3 = ght[:].rearrange("p (a k) -> p a k", k=128)
                    # replicate dstoff row via K=1 matmuls (bf16)
                    if (c - bchunks[b][0]) % DRB == 0:
                        nb = min(DRB, bchunks[b][-1] + 1 - c)
                        drt = dp_.tile([1, DRB * CHUNK], BF16, tag="drow")
                        nc.sync.dma_start(
                            out=drt[0:1, 0:nb * CHUNK],
                            in_=drow_in[0:1, c * CHUNK:(c + nb) * CHUNK])
                        sweep_bucket.drt = drt
                    drt = sweep_bucket.drt
                    doff = ((c - bchunks[b][0]) % DRB) * CHUNK
                    psr = prp.tile([128, CHUNK], F32, tag="r")
                    for hh in range(2):
                        nc.tensor.matmul(
                            out=psr[:, hh * 512:(hh + 1) * 512],
                            lhsT=ones1[:],
                            rhs=drt[0:1, doff + hh * 512:doff + (hh + 1) * 512],
                            start=True, stop=True)
                    RT = rp.tile([128, TPC * 128], F16, tag="RT")
                    nc.vector.tensor_tensor(
                        out=RT[:],
                        in0=pconst[:].to_broadcast([128, TPC * 128]),
                        in1=psr[:],
                        op=OP.is_equal)
                    # e = alpha_d[dst] + alpha_s[src]
                    psa = pap.tile([128, nl * H], F32, tag="a", name=f"psa{layer}_{c}")
                    for j in live:
                        w = tile_w[c * TPC + j]
                        nc.tensor.matmul(
                            out=psa[:, j * H:(j + 1) * H],
                            lhsT=RT[:, j * 128:(j + 1) * 128],
                            rhs=adwl[:, w * H:(w + 1) * H],
                            start=True, stop=not ew_pe)
                        if ew_pe:
                            nc.tensor.matmul(
                                out=psa[:, j * H:(j + 1) * H],
                                lhsT=identf[:],
                                rhs=ght3[:, j, D:D + H],
                                start=False, stop=True)
                    if ew_pe:
                        ewt = psa
                    else:
                        ewt = mp.tile([128, nl * H], F32, tag="ew", name=f"ew{layer}_{c}")
                        nc.vector.tensor_tensor(
                            out=ewt[:].rearrange("p (a h) -> p a h", h=H),
                            in0=psa[:].rearrange("p (a h) -> p a h", h=H),
                            in1=ght3[:, 0:nl, D:D + H],
                            op=OP.add)
                    # exp(leakyrelu(e)) = max(exp(e), exp(0.2 e))
                    z1 = mp.tile([128, nl * H], F16, tag="z1", name=f"z1_{layer}_{c}")
                    nc.scalar.activation(out=z1[:], in_=ewt[:], func=AF.Exp)
                    z2 = mp.tile([128, nl * H], F16, tag="z2", name=f"z2_{layer}_{c}")
                    nc.scalar.activation(out=z2[:], in_=ewt[:], scale=NEG, func=AF.Exp)
                    if layer == 1:
                        msgt = mp.tile([128, nl * EC], F16, tag="msg", name=f"msg1_{c}")
                        msgt3 = msgt[:].rearrange("p (a k) -> p a k", k=EC)
                        nc.vector.tensor_tensor(
                            out=msgt3[:, :, D:D + H],
                            in0=z1[:].rearrange("p (a h) -> p a h", h=H),
                            in1=z2[:].rearrange("p (a h) -> p a h", h=H),
                            op=OP.max)
                        R = rp.tile([128, TPC * 128], F16, tag="R")
                        if r_tsp:
                            for j in live:
                                nc.vector.tensor_scalar(
                                    out=R[:, j * 128:(j + 1) * 128], in0=iotac[:],
                                    scalar1=dcol_s[:, c * TPC + j:c * TPC + j + 1],
                                    scalar2=None, op0=OP.is_equal)
                        else:
                            nc.vector.tensor_tensor(
                                out=R[:].rearrange("p (a k) -> p a k", k=128),
                                in0=dcolh_s[:, c * TPC:(c + 1) * TPC, None]
                                    .to_broadcast([128, TPC, 128]),
                                in1=iotac[:, None, :].to_broadcast([128, TPC, 128]),
                                op=OP.is_equal)
                        meng = (nc.gpsimd if (msg_pool and c % 2 == 0)
                                else nc.vector)
                        meng.tensor_tensor(
                            out=msgt3[:, :, 0:D].rearrange("p a (h k) -> p a h k", k=C1),
                            in0=ght3[:, 0:nl, 0:D].rearrange("p a (h k) -> p a h k", k=C1),
                            in1=msgt3[:, :, D:D + H, None].to_broadcast(
                                [128, nl, H, C1]),
                            op=OP.mult)
                    else:
                        wt = mp.tile([128, nl], F32, tag="wt", name=f"wt_{c}")
                        nc.vector.tensor_tensor(out=wt[:], in0=z1[:], in1=z2[:],
                                                op=OP.max)
                        # ew (DVE) already consumed alpha_s col; same-engine
                        # order makes this overwrite race-free
                        nc.vector.memset(ght3[:, 0:nl, D:D + 1], 1.0)
                        # fold softmax weight into the one-hot: (iota==dst)*w
                        R = rp.tile([128, TPC * 128], F16, tag="R")
                        for j in live:
                            nc.vector.tensor_scalar(
                                out=R[:, j * 128:(j + 1) * 128], in0=iotac[:],
                                scalar1=dcol_s[:, c * TPC + j:c * TPC + j + 1],
                                scalar2=wt[:, j:j + 1], op0=OP.is_equal,
                                op1=OP.mult)
                    for j in live:
                        t = c * TPC + j
                        w = tile_w[t]
                        if tile_first[t]:
                            group_ps[w] = pgp.tile([128, EC], F32, tag="g",
                                                   name=f"grp{layer}_{w}_{b}")
                        ps = group_ps[w]
                        if layer == 1:
                            nc.tensor.matmul(
                                out=ps[:], lhsT=R[:, j * 128:(j + 1) * 128],
                                rhs=msgt[:, j * EC:(j + 1) * EC],
                                start=tile_first[t], stop=tile_last[t])
                        else:
                            nc.tensor.matmul(
                                out=ps[:], lhsT=R[:, j * 128:(j + 1) * 128],
                                rhs=ght3[:, j, 0:D + 1],
                                start=tile_first[t], stop=tile_last[t])
                        if tile_last[t]:
                            del group_ps[w]
                            nc.scalar.activation(
                                out=accb[b][:, w * EC:(w + 1) * EC],
                                in_=ps[:], func=AF.Copy)

            def clear_dead(EC):
                for w in range(NW):
                    if not wlive[w]:
                        for b in range(NBUCK):
                            nc.vector.memset(accb[b][:, w * EC:(w + 1) * EC], 0.0)

            def f1_batch(fp, fpp):
                """Softmax divide + bias + elu + transpose, WB windows at once."""
                EC = EC1
                for w0 in range(0, NW, WB):
                    cs = slice(w0 * EC, (w0 + WB) * EC)
                    sm = fp.tile([128, WB * EC], F32, tag="sm")
                    nc.vector.tensor_tensor(out=sm[:], in0=accb[0][:, cs],
                                            in1=accb[1][:, cs], op=OP.add)
                    nc.vector.tensor_tensor(out=sm[:], in0=sm[:], in1=accb[2][:, cs],
                                            op=OP.add)
                    nc.vector.tensor_tensor(out=sm[:], in0=sm[:], in1=accb[3][:, cs],
                                            op=OP.add)
                    sm3 = sm[:].rearrange("p (a k) -> p a k", k=EC)
                    den = fp.tile([128, WB * H1], F32, tag="den")
                    nc.vector.tensor_scalar_add(
                        out=den[:].rearrange("p (a h) -> p a h", h=H1),
                        in0=sm3[:, :, D:EC], scalar1=1e-16)
                    rec = fp.tile([128, WB * H1], F32, tag="rec")
                    nc.vector.reciprocal(out=rec[:], in_=den[:])
                    x2 = fp.tile([128, WB * D], F32, tag="x2")
                    nc.vector.tensor_tensor(
                        out=x2[:].rearrange("p (a h k) -> p a h k", h=H1, k=C1),
                        in0=sm3[:, :, 0:D].rearrange("p a (h k) -> p a h k", k=C1),
                        in1=rec[:].rearrange("p (a h) -> p a h", h=H1)[:, :, :, None]
                            .to_broadcast([128, WB, H1, C1]),
                        op=OP.mult)
                    nc.vector.tensor_tensor(
                        out=x2[:].rearrange("p (a k) -> p a k", k=D),
                        in0=x2[:].rearrange("p (a k) -> p a k", k=D),
                        in1=b1t[:, None, :].to_broadcast([128, WB, D]),
                        op=OP.add)
                    # elu(x) = relu(x) - relu(1 - exp(x))
                    ex = fp.tile([128, WB * D], F32, tag="ex")
                    nc.scalar.activation(out=ex[:], in_=x2[:], func=AF.Exp)
                    u = fp.tile([128, WB * D], F32, tag="u")
                    nc.scalar.activation(out=u[:], in_=ex[:], func=AF.Relu,
                                         scale=-1.0, bias=1.0)
                    r = fp.tile([128, WB * D], F32, tag="ex")
                    nc.scalar.activation(out=r[:], in_=x2[:], func=AF.Relu)
                    xe = fp.tile([128, WB * D], F32, tag="x2")
                    nc.vector.tensor_tensor(out=xe[:], in0=r[:], in1=u[:],
                                            op=OP.subtract)
                    xt = fp.tile([D, WB * 128], BF16, tag="xt")
                    for a in range(WB):
                        pst = fpp.tile([D, 128], F32, tag="t")
                        nc.tensor.transpose(out=pst[:], in_=xe[:, a * D:(a + 1) * D],
                                            identity=ident[:])
                        nc.scalar.activation(out=xt[:, a * 128:(a + 1) * 128],
                                             in_=pst[:], func=AF.Copy)
                    nc.sync.dma_start(
                        out=ag_in[:, w0 * 128:(w0 + WB) * 128], in_=xt[:])

            def f2_batch(fp):
                """Softmax divide + bias + l2-normalize, WB windows at once."""
                EC = EC2
                for w0 in range(0, NW, WB):
                    cs = slice(w0 * EC, (w0 + WB) * EC)
                    sm = fp.tile([128, WB * EC], F32, tag="sm")
                    nc.vector.tensor_tensor(out=sm[:], in0=accb[0][:, cs],
                                            in1=accb[1][:, cs], op=OP.add)
                    nc.vector.tensor_tensor(out=sm[:], in0=sm[:], in1=accb[2][:, cs],
                                            op=OP.add)
                    nc.vector.tensor_tensor(out=sm[:], in0=sm[:], in1=accb[3][:, cs],
                                            op=OP.add)
                    sm3 = sm[:].rearrange("p (a k) -> p a k", k=EC)
                    den = fp.tile([128, WB], F32, tag="den")
                    nc.vector.tensor_scalar_add(
                        out=den[:, :, None], in0=sm3[:, :, D:EC], scalar1=1e-16)
                    rec = fp.tile([128, WB], F32, tag="rec")
                    nc.vector.reciprocal(out=rec[:], in_=den[:])
                    o = fp.tile([128, WB * D], F32, tag="o")
                    nc.vector.tensor_tensor(
                        out=o[:].rearrange("p (a k) -> p a k", k=D),
                        in0=sm3[:, :, 0:D],
                        in1=rec[:, :, None].to_broadcast([128, WB, D]),
                        op=OP.mult)
                    nc.vector.tensor_tensor(
                        out=o[:].rearrange("p (a k) -> p a k", k=D),
                        in0=o[:].rearrange("p (a k) -> p a k", k=D),
                        in1=b2t[:, None, :].to_broadcast([128, WB, D]),
                        op=OP.add)
                    sq = fp.tile([128, WB * D], F32, tag="sq")
                    nc.scalar.activation(out=sq[:], in_=o[:], func=AF.Square)
                    ss = fp.tile([128, WB], F32, tag="ss")
                    nc.vector.tensor_reduce(
                        out=ss[:, :, None],
                        in_=sq[:].rearrange("p (a k) -> p a k", k=D),
                        axis=mybir.AxisListType.X, op=OP.add)
                    nrm = fp.tile([128, WB], F32, tag="nr")
                    nc.scalar.activation(out=nrm[:], in_=ss[:], func=AF.Sqrt)
                    nc.vector.tensor_scalar_max(out=nrm[:], in0=nrm[:], scalar1=1e-12)
                    rn = fp.tile([128, WB], F32, tag="rn")
                    nc.vector.reciprocal(out=rn[:], in_=nrm[:])
                    of = fp.tile([128, WB * D], F32, tag="of")
                    nc.vector.tensor_tensor(
                        out=of[:].rearrange("p (a k) -> p a k", k=D),
                        in0=o[:].rearrange("p (a k) -> p a k", k=D),
                        in1=rn[:, :, None].to_broadcast([128, WB, D]),
                        op=OP.mult)
                    nc.sync.dma_start(
                        out=out_own[w0 * 128:(w0 + WB) * 128, :].rearrange(
                            "(a p) k -> p a k", p=128),
                        in_=of[:].rearrange("p (a k) -> p a k", k=D))

            def one_pass(_rep):
                # ================= layer 1 =================
                with tc.tile_pool(name=f"l1_{_rep}", bufs=3) as dp, \
                     tc.tile_pool(name=f"l1p_{_rep}", bufs=1, space="PSUM") as dpp, \
                     tc.tile_pool(name=f"eg1_{_rep}", bufs=6) as gp, \
                     tc.tile_pool(name=f"ed1_{_rep}", bufs=2) as dp_, \
                     tc.tile_pool(name=f"er1_{_rep}", bufs=4) as rp, \
                     tc.tile_pool(name=f"em1_{_rep}", bufs=6) as mp, \
                     tc.tile_pool(name=f"epr1_{_rep}", bufs=1, space="PSUM") as prp, \
                     tc.tile_pool(name=f"epa1_{_rep}", bufs=1, space="PSUM") as pap, \
                     tc.tile_pool(name=f"epg1_{_rep}", bufs=2, space="PSUM") as pgp, \
                     tc.tile_pool(name=f"f1p_{_rep}", bufs=2, space="PSUM") as fpp:
                    with tc.tile_pool(name=f"xo1_{_rep}", bufs=1) as xop:
                        adw_fill(1, xop, pgp)
                    pools = (gp, dp_, rp, mp, prp, pap, pgp)
                    group_ps = {}
                    for b in range(NBUCK):
                        dense_bucket(1, b, dp, dpp)
                        sweep_bucket(1, b, pools, group_ps)
                    assert not group_ps
                    clear_dead(EC1)
                    if debug:
                        nc.sync.dma_start(out=dbg_adw[:], in_=adw[:])
                        for b in range(NBUCK):
                            nc.sync.dma_start(out=dbg_acc[b][:], in_=accb[b][:])
                    with tc.tile_pool(name=f"f1_{_rep}", bufs=2) as fp:
                        f1_batch(fp, fpp)
                    if debug:
                        with tc.tile_pool(name=f"dbgag_{_rep}", bufs=1) as dap:
                            agt = dap.tile([D, OWN], BF16, tag="agt")
                            nc.sync.dma_start(out=agt[:], in_=ag_in[:])
                            nc.sync.dma_start(out=dbg_ag[:], in_=agt[:])
                    # adw for layer 2 needs only own ag_in -- overlap with f1
                    with tc.tile_pool(name=f"xo2_{_rep}", bufs=1) as xop2:
                        adw_fill(2, xop2, pgp)

                if no_collective:
                    # profiling-only variant: local copy stands in for AllGather
                    with tc.tile_pool(name=f"agcp_{_rep}", bufs=2) as acp:
                        for cc in range(NCORES):
                            t_ = acp.tile([D, OWN], BF16, tag="agc")
                            nc.sync.dma_start(out=t_[:], in_=ag_in[:])
                            nc.sync.dma_start(out=ag_out[cc * D:(cc + 1) * D, :],
                                              in_=t_[:])
                else:
                    nc.gpsimd.collective_compute(
                        "AllGather", OP.bypass,
                        ins=[ag_in[:]], outs=[ag_out[:]],
                        replica_groups=[list(range(NCORES))])

                # ================= layer 2 =================
                with tc.tile_pool(name=f"l2_{_rep}", bufs=3) as dp, \
                     tc.tile_pool(name=f"l2p_{_rep}", bufs=1, space="PSUM") as dpp, \
                     tc.tile_pool(name=f"eg2_{_rep}", bufs=6) as gp, \
                     tc.tile_pool(name=f"ed2_{_rep}", bufs=2) as dp_, \
                     tc.tile_pool(name=f"er2_{_rep}", bufs=4) as rp, \
                     tc.tile_pool(name=f"em2_{_rep}", bufs=6) as mp, \
                     tc.tile_pool(name=f"epr2_{_rep}", bufs=1, space="PSUM") as prp, \
                     tc.tile_pool(name=f"epa2_{_rep}", bufs=1, space="PSUM") as pap, \
                     tc.tile_pool(name=f"epg2_{_rep}", bufs=2, space="PSUM") as pgp:
                    pools = (gp, dp_, rp, mp, prp, pap, pgp)
                    group_ps = {}
                    for b in range(NBUCK):
                        dense_bucket(2, b, dp, dpp)
                        sweep_bucket(2, b, pools, group_ps)
                    assert not group_ps
                    clear_dead(EC2)
                    if debug:
                        for b in range(NBUCK):
                            nc.sync.dma_start(out=dbg_acc2[b][:], in_=accb[b][:])
                        with tc.tile_pool(name=f"dbgao_{_rep}", bufs=2) as dap:
                            for cc in range(NCORES):
                                aot = dap.tile([D, OWN], BF16, tag="aot")
                                nc.sync.dma_start(out=aot[:],
                                                  in_=ag_out[cc * D:(cc + 1) * D, :])
                                nc.sync.dma_start(
                                    out=dbg_agout[cc * D:(cc + 1) * D, :],
                                    in_=aot[:])
                    with tc.tile_pool(name=f"f2_{_rep}", bufs=2) as fp:
                        f2_batch(fp)

            for _rep in range(repeat):
                one_pass(_rep)

    return nc


def make_inputs(edge_index, emb, W1, a_src1, a_dst1, b1, W2, a_src2, a_dst2, b2):
    NW, NPAD, NBUCK, TBL_ROWS = _derived()
    sched, idx_h, dcol_h, drow_h = prep(edge_index)

    W1 = np.asarray(W1, np.float32)
    a_s1 = np.asarray(a_src1, np.float32)
    a_d1 = np.asarray(a_dst1, np.float32)
    As = np.zeros((D, H1), np.float32)
    Ad = np.zeros((D, H1), np.float32)
    for h in range(H1):
        As[h * C1:(h + 1) * C1, h] = a_s1[h]
        Ad[h * C1:(h + 1) * C1, h] = a_d1[h]
    w1x = np.concatenate([W1, W1 @ As], 1).astype(NPBF16)
    w1d = (W1 @ Ad).astype(NPBF16)
    W2 = np.asarray(W2, np.float32)
    w2x = np.concatenate([W2, W2 @ np.asarray(a_src2, np.float32).T], 1).astype(NPBF16)
    w2d = (W2 @ np.asarray(a_dst2, np.float32).T).astype(NPBF16)

    embT = np.zeros((D, NPAD), NPBF16)
    embT[:, :N] = np.asarray(emb, np.float32).T.astype(NPBF16)
    iotac = np.broadcast_to(np.arange(128, dtype=NPF16)[None, :], (128, 128)).copy()
    pconst = np.arange(128, dtype=np.float32)[:, None].copy()
    ident = np.eye(128, dtype=np.float32)
    identf = np.eye(128, dtype=NPF16)
    ones1 = np.ones((1, 128), NPBF16)
    b1t = np.broadcast_to(np.asarray(b1, np.float32)[None, :], (128, D)).copy()
    b2t = np.broadcast_to(np.asarray(b2, np.float32)[None, :], (128, D)).copy()

    in_maps = []
    for c in range(NCORES):
        in_maps.append({
            "embT": embT, "embTo": np.ascontiguousarray(embT[:, c * OWN:(c + 1) * OWN]),
            "w1aux": w1x, "w1ad": w1d, "w2aux": w2x, "w2ad": w2d,
            "b1t": b1t, "b2t": b2t, "iotac": iotac, "pconst": pconst,
            "ident": ident, "identf": identf, "ones1": ones1,
            "idx16": idx_h[c], "dcol": dcol_h[c],
            "dcolh": dcol_h[c].astype(NPF16), "drow": drow_h[c],
        })
    return sched, in_maps


def kernel(edge_index, emb, W1, a_src1, a_dst1, b1, W2, a_src2, a_dst2, b2):
    sched, in_maps = make_inputs(edge_index, emb, W1, a_src1, a_dst1, b1,
                                 W2, a_src2, a_dst2, b2)
    nc = build(sched)
    nc.finalize()
    res = run_bass_kernel_spmd(nc, in_maps, core_ids=list(range(NCORES)))
    out = np.zeros((N, D), np.float32)
    for c in range(NCORES):
        lo, hi = c * OWN, min((c + 1) * OWN, N)
        if lo < N:
            out[lo:hi] = res.results[c]["out_own"][:hi - lo]
    return out
